# revision 35
# baseline (speedup 1.0000x reference)
"""Causal self-attention (B=4, S=2048, D=1024, H=16) on 8 Trainium2 cores.

Sharding: core c handles batch b = c//2 and head-group g = c%2 (8 heads).
Each core computes q/k/v projections for its head group, causal attention
over its 8 heads, and a partial output projection over its 512 columns of
Wo. The host sums the two per-batch partials and adds bo.

Device layouts (per core):
  x_T   [d=1024 (8x128 part-tiles), s=2048]
  q_T,k_T [e=512 (4 pair-tiles of 128 = 2 heads), s] (partition = head dim)
  v_aug [s (16 tiles of 128), 8 heads x (64 v + ones col)]
  scores_T = k_h^T-stationary matmul -> [j (keys) down, i (queries) free]
  P = exp(scale * scores_T) in bf16; diagonal blocks get trapezoid windows
  (only queries i_local >= 128k computed) + a 128-wide triangle mask.
  y~, denom = v_aug^T @ P (ones column gives the softmax denominator)
  y = y~ * bcast(1/denom); out_T[o down, s] = Wo_g^T-stationary matmul.

Scheduling: QK->exp->PV is software-pipelined per 2-j-block unit with a
depth-2 PV queue so the tensor engine never waits on the exp stream (the
ACT engine is the attention-phase co-bottleneck). All projections are
pipelined one step ahead and emitted as ~2us filler groups between
attention units, so ACT's exp backlog always overlaps tensor-only
projection work: q/k(sc,p+1) during pair p, oproj(sc-1) during pairs
1-2, v(sc+1) during pairs 2-3, q/k(sc+1,p0) during pair3. Per-pair
softmax-normalize fires from a hook when that pair's PV accumulation
drains; 1/d = exp(-ln d) runs on ACT (off the DVE queue that frees
PSUM), broadcast via a DRAM round-trip mid-kernel and via one-hot
stationary matmuls on the idle tensor engine at the tail. The last
chunk's output projection is split into two passes (pairs 0-1 early,
pairs 2-3 + SBUF-accumulated add at the end) to shorten the tail.
PSUM (16KB/partition, exact): scores ring 2x[128,1024]f32 (4 banks) +
projection ring 2x[128,512] (2) + y~ ring 2x[65,512] (2).
Input DMAs are few and partition-major (SP issue is ~0.6us each), with
pair0-only wq/wk slices first; output DMAs ride the gpsimd SW queue.
"""

import numpy as np
import ml_dtypes

import concourse.bass as bass
import concourse.mybir as mybir
import concourse.tile as tile
from concourse.vector_clock import ScopedClock

B, S, D, H = 4, 2048, 1024, 16
HD = D // H  # 64
N_CORES = 8
HG = 2  # head groups (cores per batch)
EG = D // HG  # 512 e-columns per core
PAIRS = 4  # head pairs per core
DC = D // 128  # 8 d-chunks
SC = S // 512  # 4 s/i chunks
SBLK = S // 128  # 16 s blocks
SCALE = 1.0 / float(np.sqrt(HD))

F32 = mybir.dt.float32
MM_DT = mybir.dt.bfloat16
NP_MM_DT = ml_dtypes.bfloat16

AF = mybir.ActivationFunctionType


# ---------------------------------------------------------------------------
# Workaround: this walrus build allows only ONE sync-wait on a CTRL (Drain)
# instruction; TileContext's tail drain attaches one wait per active logical
# processor. Split them across a chain of Drains (same SP program order).
def _split_drain_and_barrier(self, tick_clock, wait_clock):
    drain_inst = self.nc.sync.drain()
    wait_clock.add_sem_waits(
        drain_inst.ins, ScopedClock({None: tick_clock.global_clock})
    )
    si = drain_inst.ins.sync_info
    waits = list(si.on_wait) if si is not None and si.on_wait else []
    if len(waits) > 1:
        si.on_wait = waits[:1]
        for w in waits[1:]:
            extra = self.nc.sync.drain()
            esi = extra.ins.sync_info
            if esi is None:
                extra.ins.sync_info = mybir.SyncInfo(on_wait=[w], on_update=[])
            else:
                esi.on_wait = [w]
    self.nc.all_engine_barrier()
    assert self.sems is not None
    popped = self.nc._tile_sem_poison_stack.pop()
    assert popped is self._sem_poison
    self.nc.clear_and_free_semaphores(list(self.sems.allocated().values()))
    self.nc.all_engine_barrier()


tile.TileContext._drain_and_barrier = _split_drain_and_barrier


# The same 1-wait limit applies to every instruction class, so split any
# multi-wait instruction by hoisting excess waits onto same-engine no-fuse
# NOPs committed immediately before it (identical semantics: the engine
# executes them in order before the instruction issues).
_orig_commit = tile.TileContext._commit_instruction


def _split_commit(self, inst, lazy_reg_writes=True):
    si = getattr(inst, "sync_info", None)
    if (
        si is not None
        and si.on_wait
        and len(si.on_wait) > 1
        and getattr(inst, "engine", mybir.EngineType.Unassigned)
        != mybir.EngineType.Unassigned
    ):
        waits = list(si.on_wait)
        si.on_wait = waits[-1:]
        for w in waits[:-1]:
            nop = mybir.InstNoOp(
                name=self.nc.get_next_instruction_name(),
                engine=inst.engine,
                bass_nofuse=True,
                sync_info=mybir.SyncInfo(on_wait=[w], on_update=[]),
            )
            _orig_commit(self, nop, lazy_reg_writes=False)
    return _orig_commit(self, inst, lazy_reg_writes=lazy_reg_writes)


tile.TileContext._commit_instruction = _split_commit
# ---------------------------------------------------------------------------


def chunk_units(c):
    """j-block units for i-chunk c: list of [(b, col0, w, i_off) x2].

    b = 128-wide j-block index, col0 = column offset inside the pss tile,
    w = query-window width, i_off = query-window start within the chunk.
    Off-diagonal blocks use the full 512-query window; the 4 diagonal
    blocks use trapezoid windows (queries i_local >= 128k only).
    """
    units = []
    for jg in range(2 * c):
        units.append([(2 * jg, 0, 512, 0), (2 * jg + 1, 512, 512, 0)])
    units.append([(4 * c, 0, 512, 0), (4 * c + 1, 512, 384, 128)])
    units.append([(4 * c + 2, 0, 256, 256), (4 * c + 3, 256, 128, 384)])
    return units


def build_program():
    nc = bass.Bass()

    xT = nc.dram_tensor("xT", [DC, 128, S], MM_DT, kind="ExternalInput")
    wqT = nc.dram_tensor("wqT", [DC, 128, EG], MM_DT, kind="ExternalInput")
    wkT = nc.dram_tensor("wkT", [DC, 128, EG], MM_DT, kind="ExternalInput")
    wvT = nc.dram_tensor("wvT", [DC, 128, EG], MM_DT, kind="ExternalInput")
    woT = nc.dram_tensor("woT", [PAIRS, 128, D], MM_DT, kind="ExternalInput")
    bqd = nc.dram_tensor("bqd", [128, PAIRS], F32, kind="ExternalInput")
    bkd = nc.dram_tensor("bkd", [128, PAIRS], F32, kind="ExternalInput")
    bvaug = nc.dram_tensor("bvaug", [520], F32, kind="ExternalInput")
    seld = nc.dram_tensor("seld", [4, 256], MM_DT, kind="ExternalInput")
    outd = nc.dram_tensor("out", [DC, 128, S], MM_DT, kind="ExternalOutput")

    with tile.TileContext(nc) as tc:
        with (
            tc.tile_pool(name="const", bufs=1) as const,
            tc.tile_pool(name="big", bufs=1) as big,
            tc.tile_pool(name="ppool", bufs=5) as ppool,
            tc.tile_pool(name="ytil", bufs=8) as ytil,
            tc.tile_pool(name="dnp", bufs=2) as dnp,
            tc.tile_pool(name="rcp", bufs=2) as rcp,
            tc.tile_pool(name="bcp", bufs=2) as bcp,
            tc.tile_pool(name="drp", bufs=8, space="DRAM") as drp,
            tc.tile_pool(name="ostg", bufs=4) as ostg,
            tc.tile_pool(name="oaccp", bufs=8) as oaccp,
            tc.tile_pool(name="mmps", bufs=2, space="PSUM") as mmps,
            tc.tile_pool(name="pjps", bufs=2, space="PSUM") as pjps,
            tc.tile_pool(name="yps", bufs=2, space="PSUM") as yps,
        ):
            # ---- persistent SBUF tensors
            x_sb = big.tile([128, DC, S], MM_DT)
            wq_sb = const.tile([128, DC, EG], MM_DT)
            wk_sb = const.tile([128, DC, EG], MM_DT)
            wv_sb = const.tile([128, DC, EG], MM_DT)
            wo_sb = const.tile([128, PAIRS, D], MM_DT)
            bqs = const.tile([128, PAIRS], F32)
            bks = const.tile([128, PAIRS], F32)
            bvb = const.tile([128, 520], F32)
            q_sb = big.tile([128, PAIRS, S], MM_DT)
            k_sb = big.tile([128, PAIRS, S], MM_DT)
            v_sb = big.tile([128, SBLK, 520], MM_DT)
            y_sb = big.tile([128, PAIRS, S], MM_DT)

            # ---- input loads, staged so proj(sc=0) can start ASAP:
            # biases first (tiny), then per-dc wq/wk/wv + the first 512-col
            # x chunk, then the rest of x / wo as they're needed.
            # dc-half-granular partition-major DMAs: few enough that the
            # SP's ~0.6us-per-DMA issue cost stays small, split enough that
            # the first pq matmuls aren't gated on a whole-tensor transfer.
            def load_w(dst, srct, inner, h):
                nc.sync.dma_start(
                    dst[:, 4 * h : 4 * h + 4, :],
                    bass.AP(tensor=srct[:].tensor, offset=4 * h * 128 * inner,
                            ap=[[inner, 128], [128 * inner, 4], [1, inner]]),
                )

            def load_x_quarter(qi, h):
                nc.sync.dma_start(
                    x_sb[:, 4 * h : 4 * h + 4, 512 * qi : 512 * (qi + 1)],
                    bass.AP(tensor=xT[:].tensor,
                            offset=512 * qi + 4 * h * 128 * S,
                            ap=[[S, 128], [128 * S, 4], [1, 512]]),
                )

            def load_w_p0(dst, srct):
                # pair-0 columns only (all dc): unblocks the first q/k
                # projection ~5us before the full-tensor loads land
                nc.sync.dma_start(
                    dst[:, :, 0:128],
                    bass.AP(tensor=srct[:].tensor, offset=0,
                            ap=[[EG, 128], [128 * EG, DC], [1, 128]]),
                )

            def load_w_rest(dst, srct, h):
                # pair 1..3 columns of one dc-half (pair0 rode the preload)
                nc.sync.dma_start(
                    dst[:, 4 * h : 4 * h + 4, 128:EG],
                    bass.AP(tensor=srct[:].tensor,
                            offset=4 * h * 128 * EG + 128,
                            ap=[[EG, 128], [128 * EG, 4], [1, EG - 128]]),
                )

            load_w_p0(wq_sb, wqT)
            load_x_quarter(0, 0)
            load_x_quarter(0, 1)
            load_w_p0(wk_sb, wkT)
            load_w(wv_sb, wvT, EG, 0)
            load_w(wv_sb, wvT, EG, 1)
            load_w_rest(wq_sb, wqT, 0)
            load_w_rest(wq_sb, wqT, 1)
            load_w_rest(wk_sb, wkT, 0)
            load_w_rest(wk_sb, wkT, 1)
            nc.sync.dma_start(bqs[:], bqd[:])
            nc.sync.dma_start(bks[:], bkd[:])
            bv_ap = bvaug[:]
            nc.sync.dma_start(
                bvb[:],
                bass.AP(tensor=bv_ap.tensor, offset=bv_ap.offset,
                        ap=[[0, 128]] + list(bv_ap.ap)),
            )
            load_x_quarter(1, 0)
            load_x_quarter(1, 1)
            for h in range(2):
                nc.sync.dma_start(
                    wo_sb[:, 2 * h : 2 * h + 2, :],
                    bass.AP(tensor=woT[:].tensor, offset=2 * h * 128 * D,
                            ap=[[D, 128], [128 * D, 2], [1, D]]),
                )
            load_x_quarter(2, 0)
            load_x_quarter(2, 1)
            load_x_quarter(3, 0)
            load_x_quarter(3, 1)

            # ones columns of v_aug (col 64 of each head slot)
            v4 = v_sb[:].rearrange("p t (h c) -> p t h c", h=8)
            nc.vector.memset(v4[:, :, :, 64:65], 1.0)
            # one-hot rows for the tail's matmul-broadcast of 1/d
            sel_sb = const.tile([4, 256], MM_DT)
            nc.sync.dma_start(sel_sb[:], seld[:])

            def emit_proj_qk(sc, pair, which):
                ssl = slice(512 * sc, 512 * (sc + 1))
                psl = slice(128 * pair, 128 * (pair + 1))
                w_sb, dst, bias = (
                    (wq_sb, q_sb, bqs) if which == "q" else (wk_sb, k_sb, bks))
                pq = pjps.tile([128, 512], F32, tag="pj",
                               name=f"p{which}{sc}_{pair}")
                for dc in range(DC):
                    nc.tensor.matmul(
                        pq[:],
                        w_sb[:, dc, psl],
                        x_sb[:, dc, ssl],
                        start=(dc == 0),
                        stop=(dc == DC - 1),
                    )
                nc.vector.tensor_scalar_add(
                    dst[:, pair, ssl], pq[:], bias[:, pair : pair + 1]
                )

            def emit_proj_v(sc, sb_is=None):
                if sb_is is None:
                    sb_is = range(4 * sc, 4 * sc + 4)
                for sb_i in sb_is:
                    pv = pjps.tile([128, 512], F32, tag="pj", name=f"pv{sb_i}")
                    for dc in range(DC):
                        nc.tensor.matmul(
                            pv[:],
                            x_sb[:, dc, 128 * sb_i : 128 * (sb_i + 1)],
                            wv_sb[:, dc, :],
                            start=(dc == 0),
                            stop=(dc == DC - 1),
                        )
                    nc.vector.tensor_add(
                        v4[:, sb_i, :, 0:64],
                        pv[:].rearrange("p (h c) -> p h c", h=8),
                        bvb[:].rearrange("p (h c) -> p h c", h=8)[:, :, 0:64],
                    )

            # ---- attention machinery -------------------------------------
            pend_pv = []   # deferred PV closures (depth-2 pipeline)
            pend_yt = []   # (pair_key, closure) awaiting that pair's PV flush
            pv_left = {}   # pair_key -> # queued-but-not-emitted PV steps

            def drain_yt():
                while pend_yt and pv_left.get(pend_yt[0][0], -1) == 0:
                    pend_yt.pop(0)[1]()

            def pop_pv(n):
                drain_yt()
                for _ in range(min(n, len(pend_pv))):
                    key = pend_pv.pop(0)()
                    pv_left[key] -= 1
                    drain_yt()

            def emit_attn_unit(c, pair, unit, yp_tiles):
                """QK + exp + mask for both heads of one unit; queue the PV.

                Adjacent matmuls alternate heads so they hit disjoint PE row
                quadrants (QK) and alternate PSUM banks (PV).
                """
                key = (c, pair)
                isl0 = 512 * c
                w_tot = sum(u[2] for u in unit)
                pss = [
                    mmps.tile([128, 1024], F32, tag="mm",
                              name=f"ps{c}_{pair}_{unit[0][0]}_{a}")
                    for a in range(2)
                ]
                pts = [
                    ppool.tile([128, 1024], MM_DT, tag="pt",
                               name=f"pt{c}_{pair}_{unit[0][0]}_{a}")
                    for a in range(2)
                ]
                for b, col0, w, i_off in unit:
                    for a in range(2):
                        p0, p1 = 64 * a, 64 * a + 64
                        nc.tensor.matmul(
                            pss[a][:, col0 : col0 + w],
                            k_sb[p0:p1, pair, 128 * b : 128 * (b + 1)],
                            q_sb[p0:p1, pair, isl0 + i_off : isl0 + 512],
                        )
                for a in range(2):
                    nc.scalar.activation(
                        pts[a][:, 0:w_tot], pss[a][:, 0:w_tot], AF.Exp,
                        scale=SCALE,
                    )
                    for b, col0, w, i_off in unit:
                        if b >= 4 * c:  # diagonal: first 128 cols = triangle
                            nc.gpsimd.affine_select(
                                out=pts[a][:, col0 : col0 + 128],
                                in_=pts[a][:, col0 : col0 + 128],
                                pattern=[[1, 128]],
                                compare_op=mybir.AluOpType.is_ge,
                                fill=0.0,
                                base=0,
                                channel_multiplier=-1,
                            )

                def emit_pv():
                    for b, col0, w, i_off in unit:
                        for a in range(2):
                            hh = 2 * pair + a
                            hsl = slice(65 * hh, 65 * (hh + 1))
                            nc.tensor.matmul(
                                yp_tiles[a][:, i_off:512],
                                v_sb[:, b, hsl],
                                pts[a][:, col0 : col0 + w],
                                start=(b == 0),
                                stop=(b == 4 * c + 3),
                            )
                    return key

                pend_pv.append(emit_pv)
                pv_left[key] = pv_left.get(key, 0) + 1

            half_state = {}  # (c, half) -> [dn_tile, [(pair, yts)]]

            def make_yt_emitter(c, pair, yp_tiles):
                """Denominator gather (straight from PSUM row 64) + y~ copies;
                when the half's second pair lands: reciprocal + broadcast +
                normalize for both pairs (overlaps later pairs' attention)."""
                def go():
                    isl = slice(512 * c, 512 * (c + 1))
                    half = pair // 2
                    if c == SC - 1 and pair == 3:
                        # final pair: skip the yt->dn gather entirely. ACT
                        # reads the denominator rows straight out of PSUM
                        # (1/d = exp(-ln d) at partition 64), and a matmul
                        # against v_aug's ones-columns broadcasts it on the
                        # otherwise-idle tensor engine.
                        lnr = const.tile([65, 1024], F32, name="lnr")
                        rcr = const.tile([65, 1024], MM_DT, name="rcr")
                        bc_ps = mmps.tile([64, 1024], F32, tag="mm",
                                          name="bcp3")
                        yts = []
                        for a in range(2):
                            cs = slice(512 * a, 512 * a + 512)
                            nc.scalar.activation(
                                lnr[64:65, cs], yp_tiles[a][64:65, :], AF.Ln)
                            nc.scalar.activation(
                                rcr[64:65, cs], lnr[64:65, cs], AF.Exp,
                                scale=-1.0)
                            nc.tensor.matmul(
                                bc_ps[:, cs],
                                v4[64:65, 0:8, :, 64:65],
                                rcr[64:65, cs],
                            )
                            yt = ytil.tile([65, 512], MM_DT,
                                           name=f"yt{c}_{pair}_{a}", tag="yt")
                            nc.vector.tensor_copy(yt[0:64, :],
                                                  yp_tiles[a][0:64, :])
                            yts.append(yt)
                        for a in range(2):
                            nc.vector.tensor_mul(
                                y_sb[64 * a : 64 * a + 64, pair, isl],
                                yts[a][0:64, :],
                                bc_ps[:, 512 * a : 512 * a + 512],
                            )
                        return
                    if c == SC - 1 and half == 1:
                        # pair2 rides the overlappable DMA broadcast during
                        # pair3's attention
                        dn2 = dnp.tile([2, 512], MM_DT, name=f"dn2_{pair}",
                                       tag="dn2")
                        yts = []
                        for a in range(2):
                            yt = ytil.tile([65, 512], MM_DT,
                                           name=f"yt{c}_{pair}_{a}", tag="yt")
                            nc.vector.tensor_copy(yt[:], yp_tiles[a][:])
                            nc.sync.dma_start(dn2[a : a + 1, :], yt[64:65, :])
                            yts.append(yt)
                        lnm2 = rcp.tile([2, 512], F32, name=f"ln2_{pair}",
                                        tag="ln2")
                        nc.scalar.activation(lnm2[:], dn2[:], AF.Ln)
                        if pair == 2:
                            rc2 = rcp.tile([2, 512], F32, name="rc2", tag="rc2")
                            nc.scalar.activation(rc2[:], lnm2[:], AF.Exp,
                                                 scale=-1.0)
                            scr2 = drp.tile([2, 512], F32, name="scr2",
                                            tag="scr2")
                            nc.sync.dma_start(scr2[:], rc2[:])
                            bc = bcp.tile([64, 2, 512], F32, name="bct",
                                          tag="bc")
                            scr_ap = scr2[:]
                            nc.sync.dma_start(
                                bc[:],
                                bass.AP(tensor=scr_ap.tensor,
                                        offset=scr_ap.offset,
                                        ap=[[0, 64], [512, 2], [1, 512]]),
                            )
                            for a in range(2):
                                nc.vector.tensor_mul(
                                    y_sb[64 * a : 64 * a + 64, pair, isl],
                                    yts[a][0:64, :],
                                    bc[:, a, :],
                                )
                        else:
                            raise AssertionError("pair3 uses the direct path")
                        return
                    st = half_state.setdefault((c, half), [None, []])
                    if st[0] is None:
                        st[0] = dnp.tile([4, 512], MM_DT, name=f"dn{c}_{half}",
                                         tag="dn")
                    dn = st[0]
                    pih = pair % 2
                    yts = []
                    for a in range(2):
                        yt = ytil.tile([65, 512], MM_DT,
                                       name=f"yt{c}_{pair}_{a}", tag="yt")
                        nc.vector.tensor_copy(yt[:], yp_tiles[a][:])
                        nc.sync.dma_start(
                            dn[2 * pih + a : 2 * pih + a + 1, :],
                            yt[64:65, :],
                        )
                        yts.append(yt)
                    st[1].append((pair, yts))
                    if pih == 1:
                        # 1/d = exp(-ln d) on the ACT engine -- keeps the
                        # 3.3us iterative reciprocal off the DVE queue that
                        # frees PSUM slots for the tensor engine
                        lnm = rcp.tile([4, 512], F32, name=f"ln{c}_{half}",
                                       tag="ln")
                        nc.scalar.activation(lnm[:], dn[:], AF.Ln)
                        if c == SC - 1 and half == 1:
                            # tail: broadcast 1/d with idle-tensor matmuls
                            # (one-hot stationary) instead of the ~4us DRAM
                            # round-trip the mid-kernel path overlaps freely
                            rcb = rcp.tile([4, 512], MM_DT,
                                           name=f"rcb{c}", tag="rcb")
                            nc.scalar.activation(rcb[:], lnm[:], AF.Exp,
                                                 scale=-1.0)
                            for pr, pyts in st[1]:
                                prih = pr % 2
                                bc_ps = mmps.tile([64, 1024], F32, tag="mm",
                                                  name=f"bcp{pr}")
                                for a in range(2):
                                    k_r = 2 * prih + a
                                    nc.tensor.matmul(
                                        bc_ps[:, 512 * a : 512 * a + 512],
                                        sel_sb[:, 64 * k_r : 64 * k_r + 64],
                                        rcb[:],
                                    )
                                for a in range(2):
                                    nc.vector.tensor_mul(
                                        y_sb[64 * a : 64 * a + 64, pr, isl],
                                        pyts[a][0:64, :],
                                        bc_ps[:, 512 * a : 512 * a + 512],
                                    )
                            return
                        rc = rcp.tile([4, 512], F32, name=f"rc{c}_{half}",
                                      tag="rc")
                        nc.scalar.activation(rc[:], lnm[:], AF.Exp,
                                             scale=-1.0)
                        scr = drp.tile([4, 512], F32, name=f"scr{c}_{half}",
                                       tag="scr")
                        nc.sync.dma_start(scr[:], rc[:])
                        for pr, pyts in st[1]:
                            prih = pr % 2
                            bc = bcp.tile([64, 2, 512], F32,
                                          name=f"bc{c}_{pr}", tag="bc")
                            scr_ap = scr[:]
                            nc.sync.dma_start(
                                bc[:],
                                bass.AP(tensor=scr_ap.tensor,
                                        offset=scr_ap.offset + 512 * 2 * prih,
                                        ap=[[0, 64], [512, 2], [1, 512]]),
                            )
                            for a in range(2):
                                nc.vector.tensor_mul(
                                    y_sb[64 * a : 64 * a + 64, pr, isl],
                                    pyts[a][0:64, :],
                                    bc[:, a, :],
                                )
                return go

            def emit_oproj(c, ecs=range(PAIRS), acc=None, obs=range(DC)):
                """Output projection over e-pairs `ecs`. With acc: pass A
                stores f32 partials to SBUF; pass B adds them and stores."""
                isl = slice(512 * c, 512 * (c + 1))
                for ob in obs:
                    po = pjps.tile([128, 512], F32, tag="pj",
                                   name=f"po{c}_{ob}_{ecs[0]}")
                    for k_i, ec in enumerate(ecs):
                        nc.tensor.matmul(
                            po[:],
                            wo_sb[:, ec, 128 * ob : 128 * (ob + 1)],
                            y_sb[:, ec, isl],
                            start=(k_i == 0),
                            stop=(k_i == len(ecs) - 1),
                        )
                    if acc is not None and ecs[0] == 0:
                        oa = oaccp.tile([128, 512], F32, name=f"oacc{ob}",
                                        tag="oacc")
                        acc.append(oa)
                        nc.vector.tensor_copy(oa[:], po[:])
                        continue
                    ost = ostg.tile([128, 512], MM_DT, name=f"ost{c}_{ob}_{ecs[0]}",
                                    tag="ost")
                    if acc is not None:
                        nc.vector.tensor_add(ost[:], po[:], acc[ob][:])
                    else:
                        nc.vector.tensor_copy(ost[:], po[:])
                    nc.gpsimd.dma_start(outd[ob, :, isl], ost[:])

            # ---- main schedule -------------------------------------------
            # Projections are software-pipelined one step ahead and emitted
            # as filler groups between attention units, so the ACT engine's
            # exp stream (the attention-phase co-bottleneck) always overlaps
            # tensor-only projection work instead of running beside an idle
            # tensor engine. Layout per chunk sc (each ~2us group):
            #   pair0: q/k(sc, p1)            pair1: q/k(sc, p2) + oproj x4
            #   pair2: q/k(sc, p3) + oproj x4 + v(sc+1) x2
            #   pair3: q/k(sc+1, p0) + v(sc+1) x2   (last chunk: passA x8)
            last_acc = []
            emit_proj_qk(0, 0, "q")
            emit_proj_qk(0, 0, "k")
            for sc in range(SC):
                units = chunk_units(sc)
                pop_pv(len(pend_pv))
                for pair in range(PAIRS):
                    fillers = []
                    if sc == 0 and pair == 0:
                        fillers += [
                            (lambda sb=sb: emit_proj_v(0, sb_is=[sb]))
                            for sb in range(4)
                        ]
                    if pair < 3:
                        fillers += [
                            lambda s=sc, p=pair + 1: emit_proj_qk(s, p, "q"),
                            lambda s=sc, p=pair + 1: emit_proj_qk(s, p, "k"),
                        ]
                    if sc > 0 and pair in (1, 2):
                        obs = range(4) if pair == 1 else range(4, DC)
                        fillers += [
                            (lambda c=sc - 1, ob=ob: emit_oproj(c, obs=[ob]))
                            for ob in obs
                        ]
                    if sc < SC - 1 and pair in (2, 3):
                        sbs = (range(4 * sc + 4, 4 * sc + 6) if pair == 2
                               else range(4 * sc + 6, 4 * sc + 8))
                        fillers += [
                            (lambda s=sc + 1, sb=sb: emit_proj_v(s, sb_is=[sb]))
                            for sb in sbs
                        ]
                    if pair == 3:
                        if sc < SC - 1:
                            fillers += [
                                lambda s=sc: emit_proj_qk(s + 1, 0, "q"),
                                lambda s=sc: emit_proj_qk(s + 1, 0, "k"),
                            ]
                        else:
                            fillers += [
                                (lambda ob=ob: emit_oproj(
                                    sc, ecs=[0, 1], acc=last_acc, obs=[ob]))
                                for ob in range(DC)
                            ]
                    yp_tiles = [
                        yps.tile([65, 512], F32, name=f"yp{sc}_{pair}_{a}",
                                 tag="yp")
                        for a in range(2)
                    ]
                    pend_yt.append(
                        ((sc, pair), make_yt_emitter(sc, pair, yp_tiles))
                    )
                    for ui, unit in enumerate(units):
                        emit_attn_unit(sc, pair, unit, yp_tiles)
                        if not (sc == 0 and pair == 0):
                            left = len(units) - ui
                            n_fill = (len(fillers) + left - 1) // left
                            for _ in range(n_fill):
                                fillers.pop(0)()
                        while len(pend_pv) > 2:
                            pop_pv(1)
                    for f in fillers:
                        f()
            # tail: drain the last chunk's PVs (normalize rides the yt hooks)
            pop_pv(len(pend_pv))
            emit_oproj(SC - 1, ecs=[2, 3], acc=last_acc)

    return nc


def make_core_inputs(x, Wq, bq, Wk, bk, Wv, bv, Wo, bo, core):
    b, g = core // HG, core % HG
    esl = slice(EG * g, EG * (g + 1))
    xT = np.ascontiguousarray(x[b].T).reshape(DC, 128, S)
    wqT = np.ascontiguousarray(Wq[esl, :].T).reshape(DC, 128, EG)
    wkT = np.ascontiguousarray(Wk[esl, :].T).reshape(DC, 128, EG)
    wvT = np.ascontiguousarray(Wv[esl, :].T).reshape(DC, 128, EG)
    woT = np.ascontiguousarray(Wo[:, esl].T).reshape(PAIRS, 128, D)
    bqd = np.ascontiguousarray(bq[esl].reshape(PAIRS, 128).T)
    bkd = np.ascontiguousarray(bk[esl].reshape(PAIRS, 128).T)
    bvaug = np.zeros(520, dtype=np.float32)
    for h in range(8):
        bvaug[65 * h : 65 * h + 64] = bv[EG * g + 64 * h : EG * g + 64 * (h + 1)]
    seld = np.zeros((4, 256), dtype=NP_MM_DT)
    for k_r in range(4):
        seld[k_r, 64 * k_r : 64 * k_r + 64] = 1.0
    return {
        "xT": xT.astype(NP_MM_DT),
        "wqT": wqT.astype(NP_MM_DT),
        "wkT": wkT.astype(NP_MM_DT),
        "wvT": wvT.astype(NP_MM_DT),
        "woT": woT.astype(NP_MM_DT),
        "bqd": bqd.astype(np.float32),
        "bkd": bkd.astype(np.float32),
        "bvaug": bvaug,
        "seld": seld,
    }


_CACHED_NC = None


def kernel(x, Wq, bq, Wk, bk, Wv, bv, Wo, bo):
    global _CACHED_NC
    from concourse.bass_utils import run_bass_kernel_spmd

    x = np.asarray(x, dtype=np.float32)
    Wq = np.asarray(Wq, dtype=np.float32)
    bq = np.asarray(bq, dtype=np.float32)
    Wk = np.asarray(Wk, dtype=np.float32)
    bk = np.asarray(bk, dtype=np.float32)
    Wv = np.asarray(Wv, dtype=np.float32)
    bv = np.asarray(bv, dtype=np.float32)
    Wo = np.asarray(Wo, dtype=np.float32)
    bo = np.asarray(bo, dtype=np.float32)

    if _CACHED_NC is None:
        _CACHED_NC = build_program()
    nc = _CACHED_NC

    in_maps = [
        make_core_inputs(x, Wq, bq, Wk, bk, Wv, bv, Wo, bo, core)
        for core in range(N_CORES)
    ]
    res = run_bass_kernel_spmd(nc, in_maps, list(range(N_CORES)))

    out = np.empty((B, S, D), dtype=np.float32)
    for b in range(B):
        o0 = np.asarray(res.results[HG * b]["out"], dtype=np.float32).reshape(D, S)
        o1 = np.asarray(res.results[HG * b + 1]["out"], dtype=np.float32).reshape(D, S)
        out[b] = (o0 + o1).T + bo[None, :]
    return out


# revision 36
# speedup vs baseline: 1.0465x; 1.0465x over previous
"""Causal self-attention (B=4, S=2048, D=1024, H=16) on 8 Trainium2 cores.

Sharding: core c handles batch b = c//2 and head-group g = c%2 (8 heads).
Each core computes q/k/v projections for its head group, causal attention
over its 8 heads, and a partial output projection over its 512 columns of
Wo. The host sums the two per-batch partials and adds bo.

Device layouts (per core):
  x_T   [d=1024 (8x128 part-tiles), s=2048]
  q_T,k_T [e=512 (4 pair-tiles of 128 = 2 heads), s] (partition = head dim)
  v_aug [s (16 tiles of 128), 8 heads x (64 v + ones col)]
  scores_T = k_h^T-stationary matmul -> [j (keys) down, i (queries) free]
  P = exp(scale * scores_T) in bf16; diagonal blocks get trapezoid windows
  (only queries i_local >= 128k computed) + a 128-wide triangle mask.
  y~, denom = v_aug^T @ P (ones column gives the softmax denominator)
  y = y~ * bcast(1/denom); out_T[o down, s] = Wo_g^T-stationary matmul.

Scheduling: QK->exp->PV is software-pipelined per 2-j-block unit with a
depth-2 PV queue so the tensor engine never waits on the exp stream (the
ACT engine is the attention-phase co-bottleneck). All projections are
pipelined one step ahead and emitted as ~2us filler groups between
attention units, so ACT's exp backlog always overlaps tensor-only
projection work: q/k(sc,p+1) during pair p, oproj(sc-1) during pairs
1-2, v(sc+1) during pairs 2-3, q/k(sc+1,p0) during pair3. Per-pair
softmax-normalize fires from a hook when that pair's PV accumulation
drains; 1/d = exp(-ln d) runs on ACT (off the DVE queue that frees
PSUM), broadcast via a DRAM round-trip mid-kernel and via one-hot
stationary matmuls on the idle tensor engine at the tail. The last
chunk's output projection is split into two passes (pairs 0-1 early,
pairs 2-3 + SBUF-accumulated add at the end) to shorten the tail.
PSUM (16KB/partition, exact): scores ring 2x[128,1024]f32 (4 banks) +
projection ring 2x[128,512] (2) + y~ ring 2x[65,512] (2).
Input DMAs are few and partition-major (SP issue is ~0.6us each), with
pair0-only wq/wk slices first; output DMAs ride the gpsimd SW queue.
"""

import numpy as np
import ml_dtypes

import concourse.bass as bass
import concourse.mybir as mybir
import concourse.tile as tile
from concourse.vector_clock import ScopedClock

B, S, D, H = 4, 2048, 1024, 16
HD = D // H  # 64
N_CORES = 8
HG = 2  # head groups (cores per batch)
EG = D // HG  # 512 e-columns per core
PAIRS = 4  # head pairs per core
DC = D // 128  # 8 d-chunks
SC = S // 512  # 4 s/i chunks
SBLK = S // 128  # 16 s blocks
SCALE = 1.0 / float(np.sqrt(HD))

F32 = mybir.dt.float32
MM_DT = mybir.dt.bfloat16
NP_MM_DT = ml_dtypes.bfloat16

AF = mybir.ActivationFunctionType


# ---------------------------------------------------------------------------
# Workaround: this walrus build allows only ONE sync-wait on a CTRL (Drain)
# instruction; TileContext's tail drain attaches one wait per active logical
# processor. Split them across a chain of Drains (same SP program order).
def _split_drain_and_barrier(self, tick_clock, wait_clock):
    drain_inst = self.nc.sync.drain()
    wait_clock.add_sem_waits(
        drain_inst.ins, ScopedClock({None: tick_clock.global_clock})
    )
    si = drain_inst.ins.sync_info
    waits = list(si.on_wait) if si is not None and si.on_wait else []
    if len(waits) > 1:
        si.on_wait = waits[:1]
        for w in waits[1:]:
            extra = self.nc.sync.drain()
            esi = extra.ins.sync_info
            if esi is None:
                extra.ins.sync_info = mybir.SyncInfo(on_wait=[w], on_update=[])
            else:
                esi.on_wait = [w]
    self.nc.all_engine_barrier()
    assert self.sems is not None
    popped = self.nc._tile_sem_poison_stack.pop()
    assert popped is self._sem_poison
    self.nc.clear_and_free_semaphores(list(self.sems.allocated().values()))
    self.nc.all_engine_barrier()


tile.TileContext._drain_and_barrier = _split_drain_and_barrier


# The same 1-wait limit applies to every instruction class, so split any
# multi-wait instruction by hoisting excess waits onto same-engine no-fuse
# NOPs committed immediately before it (identical semantics: the engine
# executes them in order before the instruction issues).
_orig_commit = tile.TileContext._commit_instruction


def _split_commit(self, inst, lazy_reg_writes=True):
    si = getattr(inst, "sync_info", None)
    if (
        si is not None
        and si.on_wait
        and len(si.on_wait) > 1
        and getattr(inst, "engine", mybir.EngineType.Unassigned)
        != mybir.EngineType.Unassigned
    ):
        waits = list(si.on_wait)
        si.on_wait = waits[-1:]
        for w in waits[:-1]:
            nop = mybir.InstNoOp(
                name=self.nc.get_next_instruction_name(),
                engine=inst.engine,
                bass_nofuse=True,
                sync_info=mybir.SyncInfo(on_wait=[w], on_update=[]),
            )
            _orig_commit(self, nop, lazy_reg_writes=False)
    return _orig_commit(self, inst, lazy_reg_writes=lazy_reg_writes)


tile.TileContext._commit_instruction = _split_commit
# ---------------------------------------------------------------------------


def chunk_units(c):
    """j-block units for i-chunk c: list of [(b, col0, w, i_off) x2].

    b = 128-wide j-block index, col0 = column offset inside the pss tile,
    w = query-window width, i_off = query-window start within the chunk.
    Off-diagonal blocks use the full 512-query window; the 4 diagonal
    blocks use trapezoid windows (queries i_local >= 128k only).
    """
    units = []
    for jg in range(2 * c):
        units.append([(2 * jg, 0, 512, 0), (2 * jg + 1, 512, 512, 0)])
    units.append([(4 * c, 0, 512, 0), (4 * c + 1, 512, 384, 128)])
    units.append([(4 * c + 2, 0, 256, 256), (4 * c + 3, 256, 128, 384)])
    return units


def build_program():
    nc = bass.Bass()

    xT = nc.dram_tensor("xT", [DC, 128, S], MM_DT, kind="ExternalInput")
    wqT = nc.dram_tensor("wqT", [DC, 128, EG], MM_DT, kind="ExternalInput")
    wkT = nc.dram_tensor("wkT", [DC, 128, EG], MM_DT, kind="ExternalInput")
    wvT = nc.dram_tensor("wvT", [DC, 128, EG], MM_DT, kind="ExternalInput")
    woT = nc.dram_tensor("woT", [PAIRS, 128, D], MM_DT, kind="ExternalInput")
    bqd = nc.dram_tensor("bqd", [128, PAIRS], F32, kind="ExternalInput")
    bkd = nc.dram_tensor("bkd", [128, PAIRS], F32, kind="ExternalInput")
    bvaug = nc.dram_tensor("bvaug", [520], F32, kind="ExternalInput")
    seld = nc.dram_tensor("seld", [4, 256], MM_DT, kind="ExternalInput")
    outd = nc.dram_tensor("out", [DC, 128, S], MM_DT, kind="ExternalOutput")

    with tile.TileContext(nc) as tc:
        with (
            tc.tile_pool(name="const", bufs=1) as const,
            tc.tile_pool(name="big", bufs=1) as big,
            tc.tile_pool(name="ppool", bufs=6) as ppool,
            tc.tile_pool(name="ytil", bufs=7) as ytil,
            tc.tile_pool(name="dnp", bufs=2) as dnp,
            tc.tile_pool(name="rcp", bufs=2) as rcp,
            tc.tile_pool(name="bcp", bufs=2) as bcp,
            tc.tile_pool(name="drp", bufs=8, space="DRAM") as drp,
            tc.tile_pool(name="ostg", bufs=3) as ostg,
            tc.tile_pool(name="oaccp", bufs=8) as oaccp,
            tc.tile_pool(name="mmps", bufs=2, space="PSUM") as mmps,
            tc.tile_pool(name="pjps", bufs=2, space="PSUM") as pjps,
            tc.tile_pool(name="yps", bufs=2, space="PSUM") as yps,
        ):
            # ---- persistent SBUF tensors
            x_sb = big.tile([128, DC, S], MM_DT)
            wq_sb = const.tile([128, DC, EG], MM_DT)
            wk_sb = const.tile([128, DC, EG], MM_DT)
            wv_sb = const.tile([128, DC, EG], MM_DT)
            wo_sb = const.tile([128, PAIRS, D], MM_DT)
            bqs = const.tile([128, PAIRS], F32)
            bks = const.tile([128, PAIRS], F32)
            bvb = const.tile([128, 520], F32)
            q_sb = big.tile([128, PAIRS, S], MM_DT)
            k_sb = big.tile([128, PAIRS, S], MM_DT)
            v_sb = big.tile([128, SBLK, 520], MM_DT)
            y_sb = big.tile([128, PAIRS, S], MM_DT)

            # ---- input loads, staged so proj(sc=0) can start ASAP:
            # biases first (tiny), then per-dc wq/wk/wv + the first 512-col
            # x chunk, then the rest of x / wo as they're needed.
            # dc-half-granular partition-major DMAs: few enough that the
            # SP's ~0.6us-per-DMA issue cost stays small, split enough that
            # the first pq matmuls aren't gated on a whole-tensor transfer.
            def load_w(dst, srct, inner, h):
                nc.sync.dma_start(
                    dst[:, 4 * h : 4 * h + 4, :],
                    bass.AP(tensor=srct[:].tensor, offset=4 * h * 128 * inner,
                            ap=[[inner, 128], [128 * inner, 4], [1, inner]]),
                )

            def load_x_quarter(qi, h):
                nc.sync.dma_start(
                    x_sb[:, 4 * h : 4 * h + 4, 512 * qi : 512 * (qi + 1)],
                    bass.AP(tensor=xT[:].tensor,
                            offset=512 * qi + 4 * h * 128 * S,
                            ap=[[S, 128], [128 * S, 4], [1, 512]]),
                )

            def load_w_p0(dst, srct):
                # pair-0 columns only (all dc): unblocks the first q/k
                # projection ~5us before the full-tensor loads land
                nc.sync.dma_start(
                    dst[:, :, 0:128],
                    bass.AP(tensor=srct[:].tensor, offset=0,
                            ap=[[EG, 128], [128 * EG, DC], [1, 128]]),
                )

            def load_w_rest(dst, srct, h):
                # pair 1..3 columns of one dc-half (pair0 rode the preload)
                nc.sync.dma_start(
                    dst[:, 4 * h : 4 * h + 4, 128:EG],
                    bass.AP(tensor=srct[:].tensor,
                            offset=4 * h * 128 * EG + 128,
                            ap=[[EG, 128], [128 * EG, 4], [1, EG - 128]]),
                )

            load_w_p0(wq_sb, wqT)
            load_x_quarter(0, 0)
            load_x_quarter(0, 1)
            load_w_p0(wk_sb, wkT)
            load_w(wv_sb, wvT, EG, 0)
            load_w(wv_sb, wvT, EG, 1)
            load_w_rest(wq_sb, wqT, 0)
            load_w_rest(wq_sb, wqT, 1)
            load_w_rest(wk_sb, wkT, 0)
            load_w_rest(wk_sb, wkT, 1)
            nc.sync.dma_start(bqs[:], bqd[:])
            nc.sync.dma_start(bks[:], bkd[:])
            bv_ap = bvaug[:]
            nc.sync.dma_start(
                bvb[:],
                bass.AP(tensor=bv_ap.tensor, offset=bv_ap.offset,
                        ap=[[0, 128]] + list(bv_ap.ap)),
            )
            load_x_quarter(1, 0)
            load_x_quarter(1, 1)
            for h in range(2):
                nc.sync.dma_start(
                    wo_sb[:, 2 * h : 2 * h + 2, :],
                    bass.AP(tensor=woT[:].tensor, offset=2 * h * 128 * D,
                            ap=[[D, 128], [128 * D, 2], [1, D]]),
                )
            load_x_quarter(2, 0)
            load_x_quarter(2, 1)
            load_x_quarter(3, 0)
            load_x_quarter(3, 1)

            # ones columns of v_aug (col 64 of each head slot)
            v4 = v_sb[:].rearrange("p t (h c) -> p t h c", h=8)
            nc.vector.memset(v4[:, :, :, 64:65], 1.0)
            # one-hot rows for the tail's matmul-broadcast of 1/d
            sel_sb = const.tile([4, 256], MM_DT)
            nc.sync.dma_start(sel_sb[:], seld[:])

            def emit_proj_qk(sc, pair, which):
                ssl = slice(512 * sc, 512 * (sc + 1))
                psl = slice(128 * pair, 128 * (pair + 1))
                w_sb, dst, bias = (
                    (wq_sb, q_sb, bqs) if which == "q" else (wk_sb, k_sb, bks))
                pq = pjps.tile([128, 512], F32, tag="pj",
                               name=f"p{which}{sc}_{pair}")
                for dc in range(DC):
                    nc.tensor.matmul(
                        pq[:],
                        w_sb[:, dc, psl],
                        x_sb[:, dc, ssl],
                        start=(dc == 0),
                        stop=(dc == DC - 1),
                    )
                nc.vector.tensor_scalar_add(
                    dst[:, pair, ssl], pq[:], bias[:, pair : pair + 1]
                )

            def emit_proj_v(sc, sb_is=None):
                if sb_is is None:
                    sb_is = range(4 * sc, 4 * sc + 4)
                for sb_i in sb_is:
                    pv = pjps.tile([128, 512], F32, tag="pj", name=f"pv{sb_i}")
                    for dc in range(DC):
                        nc.tensor.matmul(
                            pv[:],
                            x_sb[:, dc, 128 * sb_i : 128 * (sb_i + 1)],
                            wv_sb[:, dc, :],
                            start=(dc == 0),
                            stop=(dc == DC - 1),
                        )
                    nc.vector.tensor_add(
                        v4[:, sb_i, :, 0:64],
                        pv[:].rearrange("p (h c) -> p h c", h=8),
                        bvb[:].rearrange("p (h c) -> p h c", h=8)[:, :, 0:64],
                    )

            # ---- attention machinery -------------------------------------
            pend_pv = []   # deferred PV closures (depth-2 pipeline)
            pend_yt = []   # (pair_key, closure) awaiting that pair's PV flush
            pv_left = {}   # pair_key -> # queued-but-not-emitted PV steps

            def drain_yt():
                while pend_yt and pv_left.get(pend_yt[0][0], -1) == 0:
                    pend_yt.pop(0)[1]()

            def pop_pv(n):
                drain_yt()
                for _ in range(min(n, len(pend_pv))):
                    key = pend_pv.pop(0)()
                    pv_left[key] -= 1
                    drain_yt()

            def emit_attn_unit(c, pair, unit, yp_tiles):
                """QK + exp + mask for both heads of one unit; queue the PV.

                Adjacent matmuls alternate heads so they hit disjoint PE row
                quadrants (QK) and alternate PSUM banks (PV).
                """
                key = (c, pair)
                isl0 = 512 * c
                w_tot = sum(u[2] for u in unit)
                pss = [
                    mmps.tile([128, 1024], F32, tag="mm",
                              name=f"ps{c}_{pair}_{unit[0][0]}_{a}")
                    for a in range(2)
                ]
                pts = [
                    ppool.tile([128, 1024], MM_DT, tag="pt",
                               name=f"pt{c}_{pair}_{unit[0][0]}_{a}")
                    for a in range(2)
                ]
                for b, col0, w, i_off in unit:
                    for a in range(2):
                        p0, p1 = 64 * a, 64 * a + 64
                        nc.tensor.matmul(
                            pss[a][:, col0 : col0 + w],
                            k_sb[p0:p1, pair, 128 * b : 128 * (b + 1)],
                            q_sb[p0:p1, pair, isl0 + i_off : isl0 + 512],
                        )
                for a in range(2):
                    nc.scalar.activation(
                        pts[a][:, 0:w_tot], pss[a][:, 0:w_tot], AF.Exp,
                        scale=SCALE,
                    )
                    for b, col0, w, i_off in unit:
                        if b >= 4 * c:  # diagonal: first 128 cols = triangle
                            nc.gpsimd.affine_select(
                                out=pts[a][:, col0 : col0 + 128],
                                in_=pts[a][:, col0 : col0 + 128],
                                pattern=[[1, 128]],
                                compare_op=mybir.AluOpType.is_ge,
                                fill=0.0,
                                base=0,
                                channel_multiplier=-1,
                            )

                def emit_pv():
                    for b, col0, w, i_off in unit:
                        for a in range(2):
                            hh = 2 * pair + a
                            hsl = slice(65 * hh, 65 * (hh + 1))
                            nc.tensor.matmul(
                                yp_tiles[a][:, i_off:512],
                                v_sb[:, b, hsl],
                                pts[a][:, col0 : col0 + w],
                                start=(b == 0),
                                stop=(b == 4 * c + 3),
                            )
                    return key

                pend_pv.append(emit_pv)
                pv_left[key] = pv_left.get(key, 0) + 1

            half_state = {}  # (c, half) -> [dn_tile, [(pair, yts)]]

            def make_yt_emitter(c, pair, yp_tiles):
                """Denominator gather (straight from PSUM row 64) + y~ copies;
                when the half's second pair lands: reciprocal + broadcast +
                normalize for both pairs (overlaps later pairs' attention)."""
                def go():
                    isl = slice(512 * c, 512 * (c + 1))
                    half = pair // 2
                    if c == SC - 1 and pair == 3:
                        # final pair: skip the yt->dn gather entirely. ACT
                        # reads the denominator rows straight out of PSUM
                        # (1/d = exp(-ln d) at partition 64), and a matmul
                        # against v_aug's ones-columns broadcasts it on the
                        # otherwise-idle tensor engine.
                        lnr = const.tile([65, 1024], F32, name="lnr")
                        rcr = const.tile([65, 1024], MM_DT, name="rcr")
                        bc_ps = mmps.tile([64, 1024], F32, tag="mm",
                                          name="bcp3")
                        yts = []
                        for a in range(2):
                            cs = slice(512 * a, 512 * a + 512)
                            nc.scalar.activation(
                                lnr[64:65, cs], yp_tiles[a][64:65, :], AF.Ln)
                            nc.scalar.activation(
                                rcr[64:65, cs], lnr[64:65, cs], AF.Exp,
                                scale=-1.0)
                            nc.tensor.matmul(
                                bc_ps[:, cs],
                                v4[64:65, 0:8, :, 64:65],
                                rcr[64:65, cs],
                            )
                            yt = ytil.tile([65, 512], MM_DT,
                                           name=f"yt{c}_{pair}_{a}", tag="yt")
                            nc.vector.tensor_copy(yt[0:64, :],
                                                  yp_tiles[a][0:64, :])
                            yts.append(yt)
                        for a in range(2):
                            nc.vector.tensor_mul(
                                y_sb[64 * a : 64 * a + 64, pair, isl],
                                yts[a][0:64, :],
                                bc_ps[:, 512 * a : 512 * a + 512],
                            )
                        return
                    if c == SC - 1 and half == 1:
                        # pair2 rides the overlappable DMA broadcast during
                        # pair3's attention
                        dn2 = dnp.tile([2, 512], MM_DT, name=f"dn2_{pair}",
                                       tag="dn2")
                        yts = []
                        for a in range(2):
                            yt = ytil.tile([65, 512], MM_DT,
                                           name=f"yt{c}_{pair}_{a}", tag="yt")
                            nc.vector.tensor_copy(yt[:], yp_tiles[a][:])
                            nc.sync.dma_start(dn2[a : a + 1, :], yt[64:65, :])
                            yts.append(yt)
                        lnm2 = rcp.tile([2, 512], F32, name=f"ln2_{pair}",
                                        tag="ln2")
                        nc.scalar.activation(lnm2[:], dn2[:], AF.Ln)
                        if pair == 2:
                            rc2 = rcp.tile([2, 512], F32, name="rc2", tag="rc2")
                            nc.scalar.activation(rc2[:], lnm2[:], AF.Exp,
                                                 scale=-1.0)
                            scr2 = drp.tile([2, 512], F32, name="scr2",
                                            tag="scr2")
                            nc.sync.dma_start(scr2[:], rc2[:])
                            bc = bcp.tile([64, 2, 512], F32, name="bct",
                                          tag="bc")
                            scr_ap = scr2[:]
                            nc.sync.dma_start(
                                bc[:],
                                bass.AP(tensor=scr_ap.tensor,
                                        offset=scr_ap.offset,
                                        ap=[[0, 64], [512, 2], [1, 512]]),
                            )
                            for a in range(2):
                                nc.vector.tensor_mul(
                                    y_sb[64 * a : 64 * a + 64, pair, isl],
                                    yts[a][0:64, :],
                                    bc[:, a, :],
                                )
                        else:
                            raise AssertionError("pair3 uses the direct path")
                        return
                    st = half_state.setdefault((c, half), [None, []])
                    if st[0] is None:
                        st[0] = dnp.tile([4, 512], MM_DT, name=f"dn{c}_{half}",
                                         tag="dn")
                    dn = st[0]
                    pih = pair % 2
                    yts = []
                    for a in range(2):
                        yt = ytil.tile([65, 512], MM_DT,
                                       name=f"yt{c}_{pair}_{a}", tag="yt")
                        nc.vector.tensor_copy(yt[:], yp_tiles[a][:])
                        nc.sync.dma_start(
                            dn[2 * pih + a : 2 * pih + a + 1, :],
                            yt[64:65, :],
                        )
                        yts.append(yt)
                    st[1].append((pair, yts))
                    if pih == 1:
                        # 1/d = exp(-ln d) on the ACT engine -- keeps the
                        # 3.3us iterative reciprocal off the DVE queue that
                        # frees PSUM slots for the tensor engine
                        lnm = rcp.tile([4, 512], F32, name=f"ln{c}_{half}",
                                       tag="ln")
                        nc.scalar.activation(lnm[:], dn[:], AF.Ln)
                        if c == SC - 1 and half == 1:
                            # tail: broadcast 1/d with idle-tensor matmuls
                            # (one-hot stationary) instead of the ~4us DRAM
                            # round-trip the mid-kernel path overlaps freely
                            rcb = rcp.tile([4, 512], MM_DT,
                                           name=f"rcb{c}", tag="rcb")
                            nc.scalar.activation(rcb[:], lnm[:], AF.Exp,
                                                 scale=-1.0)
                            for pr, pyts in st[1]:
                                prih = pr % 2
                                bc_ps = mmps.tile([64, 1024], F32, tag="mm",
                                                  name=f"bcp{pr}")
                                for a in range(2):
                                    k_r = 2 * prih + a
                                    nc.tensor.matmul(
                                        bc_ps[:, 512 * a : 512 * a + 512],
                                        sel_sb[:, 64 * k_r : 64 * k_r + 64],
                                        rcb[:],
                                    )
                                for a in range(2):
                                    nc.vector.tensor_mul(
                                        y_sb[64 * a : 64 * a + 64, pr, isl],
                                        pyts[a][0:64, :],
                                        bc_ps[:, 512 * a : 512 * a + 512],
                                    )
                            return
                        rc = rcp.tile([4, 512], F32, name=f"rc{c}_{half}",
                                      tag="rc")
                        nc.scalar.activation(rc[:], lnm[:], AF.Exp,
                                             scale=-1.0)
                        scr = drp.tile([4, 512], F32, name=f"scr{c}_{half}",
                                       tag="scr")
                        nc.sync.dma_start(scr[:], rc[:])
                        for pr, pyts in st[1]:
                            prih = pr % 2
                            bc = bcp.tile([64, 2, 512], F32,
                                          name=f"bc{c}_{pr}", tag="bc")
                            scr_ap = scr[:]
                            nc.sync.dma_start(
                                bc[:],
                                bass.AP(tensor=scr_ap.tensor,
                                        offset=scr_ap.offset + 512 * 2 * prih,
                                        ap=[[0, 64], [512, 2], [1, 512]]),
                            )
                            for a in range(2):
                                nc.vector.tensor_mul(
                                    y_sb[64 * a : 64 * a + 64, pr, isl],
                                    pyts[a][0:64, :],
                                    bc[:, a, :],
                                )
                return go

            def emit_oproj(c, ecs=range(PAIRS), acc=None, obs=range(DC)):
                """Output projection over e-pairs `ecs`. With acc: pass A
                stores f32 partials to SBUF; pass B adds them and stores."""
                isl = slice(512 * c, 512 * (c + 1))
                for ob in obs:
                    po = pjps.tile([128, 512], F32, tag="pj",
                                   name=f"po{c}_{ob}_{ecs[0]}")
                    for k_i, ec in enumerate(ecs):
                        nc.tensor.matmul(
                            po[:],
                            wo_sb[:, ec, 128 * ob : 128 * (ob + 1)],
                            y_sb[:, ec, isl],
                            start=(k_i == 0),
                            stop=(k_i == len(ecs) - 1),
                        )
                    if acc is not None and ecs[0] == 0:
                        oa = oaccp.tile([128, 512], F32, name=f"oacc{ob}",
                                        tag="oacc")
                        acc.append(oa)
                        nc.vector.tensor_copy(oa[:], po[:])
                        continue
                    ost = ostg.tile([128, 512], MM_DT, name=f"ost{c}_{ob}_{ecs[0]}",
                                    tag="ost")
                    if acc is not None:
                        nc.vector.tensor_add(ost[:], po[:], acc[ob][:])
                    else:
                        nc.vector.tensor_copy(ost[:], po[:])
                    nc.gpsimd.dma_start(outd[ob, :, isl], ost[:])

            # ---- main schedule -------------------------------------------
            # Projections are software-pipelined one step ahead and emitted
            # as filler groups between attention units, so the ACT engine's
            # exp stream (the attention-phase co-bottleneck) always overlaps
            # tensor-only projection work instead of running beside an idle
            # tensor engine. Layout per chunk sc (each ~2us group):
            #   pair0: q/k(sc, p1)            pair1: q/k(sc, p2) + oproj x4
            #   pair2: q/k(sc, p3) + oproj x4 + v(sc+1) x2
            #   pair3: q/k(sc+1, p0) + v(sc+1) x2   (last chunk: passA x8)
            last_acc = []
            emit_proj_qk(0, 0, "q")
            emit_proj_qk(0, 0, "k")
            for sc in range(SC):
                units = chunk_units(sc)
                pop_pv(len(pend_pv))
                for pair in range(PAIRS):
                    fillers = []
                    if sc == 0 and pair == 0:
                        fillers += [
                            (lambda sb=sb: emit_proj_v(0, sb_is=[sb]))
                            for sb in range(4)
                        ]
                    if pair < 3:
                        fillers += [
                            lambda s=sc, p=pair + 1: emit_proj_qk(s, p, "q"),
                            lambda s=sc, p=pair + 1: emit_proj_qk(s, p, "k"),
                        ]
                    if sc > 0 and pair in (1, 2):
                        obs = range(4) if pair == 1 else range(4, DC)
                        fillers += [
                            (lambda c=sc - 1, ob=ob: emit_oproj(c, obs=[ob]))
                            for ob in obs
                        ]
                    if sc < SC - 1 and pair in (2, 3):
                        sbs = (range(4 * sc + 4, 4 * sc + 6) if pair == 2
                               else range(4 * sc + 6, 4 * sc + 8))
                        fillers += [
                            (lambda s=sc + 1, sb=sb: emit_proj_v(s, sb_is=[sb]))
                            for sb in sbs
                        ]
                    if pair == 3:
                        if sc < SC - 1:
                            fillers += [
                                lambda s=sc: emit_proj_qk(s + 1, 0, "q"),
                                lambda s=sc: emit_proj_qk(s + 1, 0, "k"),
                            ]
                        else:
                            fillers += [
                                (lambda ob=ob: emit_oproj(
                                    sc, ecs=[0, 1], acc=last_acc, obs=[ob]))
                                for ob in range(DC)
                            ]
                    yp_tiles = [
                        yps.tile([65, 512], F32, name=f"yp{sc}_{pair}_{a}",
                                 tag="yp")
                        for a in range(2)
                    ]
                    pend_yt.append(
                        ((sc, pair), make_yt_emitter(sc, pair, yp_tiles))
                    )
                    for ui, unit in enumerate(units):
                        emit_attn_unit(sc, pair, unit, yp_tiles)
                        if not (sc == 0 and pair == 0):
                            left = len(units) - ui
                            n_fill = (len(fillers) + left - 1) // left
                            for _ in range(n_fill):
                                fillers.pop(0)()
                        while len(pend_pv) > 2:
                            pop_pv(1)
                    for f in fillers:
                        f()
            # tail: drain the last chunk's PVs (normalize rides the yt hooks)
            pop_pv(len(pend_pv))
            emit_oproj(SC - 1, ecs=[2, 3], acc=last_acc)

    return nc


def make_core_inputs(x, Wq, bq, Wk, bk, Wv, bv, Wo, bo, core):
    b, g = core // HG, core % HG
    esl = slice(EG * g, EG * (g + 1))
    xT = np.ascontiguousarray(x[b].T).reshape(DC, 128, S)
    wqT = np.ascontiguousarray(Wq[esl, :].T).reshape(DC, 128, EG)
    wkT = np.ascontiguousarray(Wk[esl, :].T).reshape(DC, 128, EG)
    wvT = np.ascontiguousarray(Wv[esl, :].T).reshape(DC, 128, EG)
    woT = np.ascontiguousarray(Wo[:, esl].T).reshape(PAIRS, 128, D)
    bqd = np.ascontiguousarray(bq[esl].reshape(PAIRS, 128).T)
    bkd = np.ascontiguousarray(bk[esl].reshape(PAIRS, 128).T)
    bvaug = np.zeros(520, dtype=np.float32)
    for h in range(8):
        bvaug[65 * h : 65 * h + 64] = bv[EG * g + 64 * h : EG * g + 64 * (h + 1)]
    seld = np.zeros((4, 256), dtype=NP_MM_DT)
    for k_r in range(4):
        seld[k_r, 64 * k_r : 64 * k_r + 64] = 1.0
    return {
        "xT": xT.astype(NP_MM_DT),
        "wqT": wqT.astype(NP_MM_DT),
        "wkT": wkT.astype(NP_MM_DT),
        "wvT": wvT.astype(NP_MM_DT),
        "woT": woT.astype(NP_MM_DT),
        "bqd": bqd.astype(np.float32),
        "bkd": bkd.astype(np.float32),
        "bvaug": bvaug,
        "seld": seld,
    }


_CACHED_NC = None


def kernel(x, Wq, bq, Wk, bk, Wv, bv, Wo, bo):
    global _CACHED_NC
    from concourse.bass_utils import run_bass_kernel_spmd

    x = np.asarray(x, dtype=np.float32)
    Wq = np.asarray(Wq, dtype=np.float32)
    bq = np.asarray(bq, dtype=np.float32)
    Wk = np.asarray(Wk, dtype=np.float32)
    bk = np.asarray(bk, dtype=np.float32)
    Wv = np.asarray(Wv, dtype=np.float32)
    bv = np.asarray(bv, dtype=np.float32)
    Wo = np.asarray(Wo, dtype=np.float32)
    bo = np.asarray(bo, dtype=np.float32)

    if _CACHED_NC is None:
        _CACHED_NC = build_program()
    nc = _CACHED_NC

    in_maps = [
        make_core_inputs(x, Wq, bq, Wk, bk, Wv, bv, Wo, bo, core)
        for core in range(N_CORES)
    ]
    res = run_bass_kernel_spmd(nc, in_maps, list(range(N_CORES)))

    out = np.empty((B, S, D), dtype=np.float32)
    for b in range(B):
        o0 = np.asarray(res.results[HG * b]["out"], dtype=np.float32).reshape(D, S)
        o1 = np.asarray(res.results[HG * b + 1]["out"], dtype=np.float32).reshape(D, S)
        out[b] = (o0 + o1).T + bo[None, :]
    return out
